# revision 19
# baseline (speedup 1.0000x reference)
"""Balanced dice loss (histogram binning) on 8 Trainium2 NeuronCores.

Math: with t ∈ {0,1} and p = sigmoid(x), the reference loss only needs
four global sums:
    S_t   = Σ t            (count of ones — the bincount)
    S_pt  = Σ p·t
    S_pp  = Σ p²
    S_ppt = Σ p²·t
Then with c1 = S_t, c0 = N − c1, w0 = 1/(c0+s)², w1 = 1/(c1+s)²:
    intersection = w1·S_pt
    denominator  = w0·(S_pp − S_ppt) + w1·(S_ppt + c1)
    dice = 1 − (2·I + s)/(D + s)

Device kernel (data-parallel over 8 cores, batch-sharded), per [128,F] tile:
    ACT : p = sigmoid(x) (bf16); float(t) copy with row-accum → S_t
    DVE : u = p·t, w = u·p (= p²t), sq = p·p   (bf16 products)
    PE  : ones[128,128] @ chunks of u/w/sq → PSUM column-sum accumulation
          (full-array weights so the matmul streams at ~1 col/cycle)
Per-partition/per-tile partials are DMA'd out; host reduces in float64.
"""

import numpy as np

import concourse.bacc as bacc
import concourse.mybir as mybir
from concourse.bass_utils import run_bass_kernel_spmd
from concourse.tile import TileContext

N_CORES = 8
P = 128
TOTAL = 32 * 1024 * 1024  # elements in the full problem
PER_CORE = TOTAL // N_CORES  # 4,194,304
FREE = PER_CORE // P  # 32,768 f32 per partition
F = 4096  # tile free-dim
NT = FREE // F  # tiles per core
MMN = 512  # matmul moving free-dim (one PSUM bank)
FA = 1536  # S_t split: first FA columns summed on ACT, rest on DVE
NCH = F // MMN  # matmul chunks per tile
SMOOTH = 1e-05

_nc_cache = None


def _build_bass():
    nc = bacc.Bacc(None, target_bir_lowering=False)
    x = nc.dram_tensor("input", [P, FREE], mybir.dt.float32, kind="ExternalInput")
    t = nc.dram_tensor("target", [P, FREE], mybir.dt.int32, kind="ExternalInput")
    o_pt = nc.dram_tensor("o_pt", [1, 2 * MMN], mybir.dt.float32, kind="ExternalOutput")
    o_ppt = nc.dram_tensor("o_ppt", [1, 2 * MMN], mybir.dt.float32, kind="ExternalOutput")
    o_pp = nc.dram_tensor("o_pp", [P, NT], mybir.dt.float32, kind="ExternalOutput")
    o_t = nc.dram_tensor("o_t", [P, NT], mybir.dt.float32, kind="ExternalOutput")
    o_t2 = nc.dram_tensor("o_t2", [P, NT], mybir.dt.int32, kind="ExternalOutput")

    with TileContext(nc) as tc:
        with (
            tc.tile_pool(name="work", bufs=2) as pool,
            tc.tile_pool(name="stats", bufs=1) as spool,
            tc.tile_pool(name="ps", bufs=1, space="PSUM") as psum,
        ):
            s_t = spool.tile([P, NT], mybir.dt.float32)
            s_t2 = spool.tile([P, NT], mybir.dt.int32)
            s_pp = spool.tile([P, NT], mybir.dt.float32)
            junk = spool.tile([P, F], mybir.dt.float32, tag="junk")
            ones = spool.tile([P, P], mybir.dt.bfloat16, tag="ones")
            ps_pt_a = psum.tile([P, MMN], mybir.dt.float32, tag="ps_pt_a")
            ps_pt_b = psum.tile([P, MMN], mybir.dt.float32, tag="ps_pt_b")
            ps_ppt_a = psum.tile([P, MMN], mybir.dt.float32, tag="ps_ppt_a")
            ps_ppt_b = psum.tile([P, MMN], mybir.dt.float32, tag="ps_ppt_b")
            nc.any.memset(ones, 1.0)

            for i in range(NT):
                xt = pool.tile([P, F], mybir.dt.float32, tag="xt", bufs=3)
                tt = pool.tile([P, F], mybir.dt.int32, tag="tt", bufs=3)
                pt_ = pool.tile([P, F], mybir.dt.bfloat16, tag="p")
                u = pool.tile([P, F], mybir.dt.bfloat16, tag="u", bufs=3)
                w = pool.tile([P, F], mybir.dt.bfloat16, tag="w", bufs=3)

                nc.sync.dma_start(xt[:], x[:, i * F : (i + 1) * F])
                nc.sync.dma_start(tt[:], t[:, i * F : (i + 1) * F])

                # p = sigmoid(x)                                   [ACT]
                nc.scalar.activation(
                    pt_[:], xt[:], mybir.ActivationFunctionType.Sigmoid
                )
                # u = p·t (bf16·int32), w = u·p                    [DVE]
                nc.vector.tensor_tensor(
                    out=u[:], in0=pt_[:], in1=tt[:], op=mybir.AluOpType.mult
                )
                nc.vector.tensor_tensor(
                    out=w[:], in0=u[:], in1=pt_[:], op=mybir.AluOpType.mult
                )
                # S_pp row-accum via p² (junk sink)                [ACT]
                nc.scalar.activation(
                    junk[:],
                    pt_[:],
                    mybir.ActivationFunctionType.Square,
                    accum_out=s_pp[:, i : i + 1],
                )
                # S_t split: float(t) copy+accum on [:FA]          [ACT]
                nc.scalar.activation(
                    junk[:, :FA],
                    tt[:, :FA],
                    mybir.ActivationFunctionType.Copy,
                    accum_out=s_t[:, i : i + 1],
                )
                # ... and int32 row-reduce on [FA:] (exact)         [DVE]
                with nc.allow_low_precision("int32 add is exact"):
                    nc.vector.tensor_reduce(
                        s_t2[:, i : i + 1],
                        tt[:, FA:],
                        axis=mybir.AxisListType.X,
                        op=mybir.AluOpType.add,
                    )
                # column-sum accumulation of u and w; each chain
                # alternates two PSUM banks to pipeline the RMW    [PE]
                for s_, banks in ((u, (ps_pt_a, ps_pt_b)), (w, (ps_ppt_a, ps_ppt_b))):
                    for j in range(NCH):
                        nc.tensor.matmul(
                            banks[j % 2][:],
                            ones[:],
                            s_[:, j * MMN : (j + 1) * MMN],
                            start=(i == 0 and j < 2),
                            stop=(i == NT - 1 and j >= NCH - 2),
                        )

            fin = spool.tile([1, 4 * MMN], mybir.dt.float32, tag="fin")
            nc.vector.tensor_copy(fin[:, 0:MMN], ps_pt_a[0:1, :])
            nc.vector.tensor_copy(fin[:, MMN : 2 * MMN], ps_pt_b[0:1, :])
            nc.vector.tensor_copy(fin[:, 2 * MMN : 3 * MMN], ps_ppt_a[0:1, :])
            nc.vector.tensor_copy(fin[:, 3 * MMN :], ps_ppt_b[0:1, :])
            nc.sync.dma_start(o_pt[:], fin[:, 0 : 2 * MMN])
            nc.sync.dma_start(o_ppt[:], fin[:, 2 * MMN :])
            nc.sync.dma_start(o_pp[:], s_pp[:])
            nc.sync.dma_start(o_t[:], s_t[:])
            nc.sync.dma_start(o_t2[:], s_t2[:])
    nc.finalize()
    return nc


def _get_nc():
    global _nc_cache
    if _nc_cache is None:
        _nc_cache = _build_bass()
    return _nc_cache


def kernel(input, target, _trace=False):
    x = np.ascontiguousarray(np.asarray(input, dtype=np.float32)).reshape(
        N_CORES, P, FREE
    )
    t = np.ascontiguousarray(np.asarray(target, dtype=np.int32)).reshape(
        N_CORES, P, FREE
    )
    in_maps = [{"input": x[i], "target": t[i]} for i in range(N_CORES)]

    nc = _get_nc()
    res = run_bass_kernel_spmd(
        nc, in_maps, core_ids=list(range(N_CORES)), trace=_trace
    )
    kernel.last_results = res

    s_pt = s_ppt = s_pp = s_t = 0.0
    for r in res.results:
        s_pt += float(r["o_pt"].astype(np.float64).sum())
        s_ppt += float(r["o_ppt"].astype(np.float64).sum())
        s_pp += float(r["o_pp"].astype(np.float64).sum())
        s_t += float(r["o_t"].astype(np.float64).sum())
        s_t += float(r["o_t2"].astype(np.int64).sum())

    c1 = float(s_t)
    c0 = float(TOTAL - s_t)
    w0 = 1.0 / (c0 + SMOOTH) ** 2
    w1 = 1.0 / (c1 + SMOOTH) ** 2
    intersection = w1 * s_pt
    denominator = w0 * (s_pp - s_ppt) + w1 * (s_ppt + c1)
    dice = 1.0 - (2.0 * intersection + SMOOTH) / (denominator + SMOOTH)
    return np.asarray(dice, dtype=np.float32)


# revision 20
# speedup vs baseline: 1.0169x; 1.0169x over previous
"""Balanced dice loss (histogram binning) on 8 Trainium2 NeuronCores.

Math: with t ∈ {0,1} and p = sigmoid(x), the reference loss only needs
four global sums:
    S_t   = Σ t            (count of ones — the bincount)
    S_pt  = Σ p·t
    S_pp  = Σ p²
    S_ppt = Σ p²·t
Then with c1 = S_t, c0 = N − c1, w0 = 1/(c0+s)², w1 = 1/(c1+s)²:
    intersection = w1·S_pt
    denominator  = w0·(S_pp − S_ppt) + w1·(S_ppt + c1)
    dice = 1 − (2·I + s)/(D + s)

Device kernel (data-parallel over 8 cores, batch-sharded), per [128,F] tile:
    ACT : p = sigmoid(x) (bf16); float(t) copy with row-accum → S_t
    DVE : u = p·t, w = u·p (= p²t), sq = p·p   (bf16 products)
    PE  : ones[128,128] @ chunks of u/w/sq → PSUM column-sum accumulation
          (full-array weights so the matmul streams at ~1 col/cycle)
Per-partition/per-tile partials are DMA'd out; host reduces in float64.
"""

import numpy as np

import concourse.bacc as bacc
import concourse.mybir as mybir
from concourse.bass_utils import run_bass_kernel_spmd
from concourse.tile import TileContext

N_CORES = 8
P = 128
TOTAL = 32 * 1024 * 1024  # elements in the full problem
PER_CORE = TOTAL // N_CORES  # 4,194,304
FREE = PER_CORE // P  # 32,768 f32 per partition
F = 4096  # tile free-dim
NT = FREE // F  # tiles per core
MMN = 512  # matmul moving free-dim (one PSUM bank)
FA = 1536  # S_t split: first FA columns summed on ACT, rest on DVE
NCH = F // MMN  # matmul chunks per tile
SMOOTH = 1e-05

_nc_cache = None


def _build_bass():
    nc = bacc.Bacc(None, target_bir_lowering=False)
    x = nc.dram_tensor("input", [P, FREE], mybir.dt.float32, kind="ExternalInput")
    t = nc.dram_tensor("target", [P, FREE], mybir.dt.int32, kind="ExternalInput")
    o_pt = nc.dram_tensor("o_pt", [1, 2 * MMN], mybir.dt.float32, kind="ExternalOutput")
    o_ppt = nc.dram_tensor("o_ppt", [1, 2 * MMN], mybir.dt.float32, kind="ExternalOutput")
    o_pp = nc.dram_tensor("o_pp", [P, NT], mybir.dt.float32, kind="ExternalOutput")
    o_t = nc.dram_tensor("o_t", [P, NT], mybir.dt.float32, kind="ExternalOutput")
    o_t2 = nc.dram_tensor("o_t2", [P, NT], mybir.dt.int32, kind="ExternalOutput")

    with TileContext(nc) as tc:
        with (
            tc.tile_pool(name="work", bufs=2) as pool,
            tc.tile_pool(name="stats", bufs=1) as spool,
            tc.tile_pool(name="ps", bufs=1, space="PSUM") as psum,
        ):
            s_t = spool.tile([P, NT], mybir.dt.float32)
            s_t2 = spool.tile([P, NT], mybir.dt.int32)
            s_pp = spool.tile([P, NT], mybir.dt.float32)
            junk = spool.tile([P, F], mybir.dt.float32, tag="junk")
            ones = spool.tile([P, P], mybir.dt.bfloat16, tag="ones")
            ps_pt_a = psum.tile([P, MMN], mybir.dt.float32, tag="ps_pt_a")
            ps_pt_b = psum.tile([P, MMN], mybir.dt.float32, tag="ps_pt_b")
            ps_ppt_a = psum.tile([P, MMN], mybir.dt.float32, tag="ps_ppt_a")
            ps_ppt_b = psum.tile([P, MMN], mybir.dt.float32, tag="ps_ppt_b")
            nc.any.memset(ones, 1.0)

            for i in range(NT):
                xt = pool.tile([P, F], mybir.dt.float32, tag="xt", bufs=3)
                tt = pool.tile([P, F], mybir.dt.int32, tag="tt", bufs=3)
                pt_ = pool.tile([P, F], mybir.dt.bfloat16, tag="p")
                u = pool.tile([P, F], mybir.dt.bfloat16, tag="u")
                w = pool.tile([P, F], mybir.dt.bfloat16, tag="w")

                nc.sync.dma_start(xt[:], x[:, i * F : (i + 1) * F])
                nc.sync.dma_start(tt[:], t[:, i * F : (i + 1) * F])

                # p = sigmoid(x)                                   [ACT]
                nc.scalar.activation(
                    pt_[:], xt[:], mybir.ActivationFunctionType.Sigmoid
                )
                # u = p·t (bf16·int32), w = u·p                    [DVE]
                nc.vector.tensor_tensor(
                    out=u[:], in0=pt_[:], in1=tt[:], op=mybir.AluOpType.mult
                )
                nc.vector.tensor_tensor(
                    out=w[:], in0=u[:], in1=pt_[:], op=mybir.AluOpType.mult
                )
                # S_pp row-accum via p² (junk sink)                [ACT]
                nc.scalar.activation(
                    junk[:],
                    pt_[:],
                    mybir.ActivationFunctionType.Square,
                    accum_out=s_pp[:, i : i + 1],
                )
                # S_t split: float(t) copy+accum on [:FA]          [ACT]
                nc.scalar.activation(
                    junk[:, :FA],
                    tt[:, :FA],
                    mybir.ActivationFunctionType.Copy,
                    accum_out=s_t[:, i : i + 1],
                )
                # ... and int32 row-reduce on [FA:] (exact)         [DVE]
                with nc.allow_low_precision("int32 add is exact"):
                    nc.vector.tensor_reduce(
                        s_t2[:, i : i + 1],
                        tt[:, FA:],
                        axis=mybir.AxisListType.X,
                        op=mybir.AluOpType.add,
                    )
                # column-sum accumulation of u and w; each chain
                # alternates two PSUM banks to pipeline the RMW    [PE]
                for s_, banks in ((u, (ps_pt_a, ps_pt_b)), (w, (ps_ppt_a, ps_ppt_b))):
                    for j in range(NCH):
                        nc.tensor.matmul(
                            banks[j % 2][:],
                            ones[:],
                            s_[:, j * MMN : (j + 1) * MMN],
                            start=(i == 0 and j < 2),
                            stop=(i == NT - 1 and j >= NCH - 2),
                        )

            fin = spool.tile([1, 4 * MMN], mybir.dt.float32, tag="fin")
            nc.vector.tensor_copy(fin[:, 0:MMN], ps_pt_a[0:1, :])
            nc.vector.tensor_copy(fin[:, MMN : 2 * MMN], ps_pt_b[0:1, :])
            nc.vector.tensor_copy(fin[:, 2 * MMN : 3 * MMN], ps_ppt_a[0:1, :])
            nc.vector.tensor_copy(fin[:, 3 * MMN :], ps_ppt_b[0:1, :])
            nc.sync.dma_start(o_pt[:], fin[:, 0 : 2 * MMN])
            nc.sync.dma_start(o_ppt[:], fin[:, 2 * MMN :])
            nc.sync.dma_start(o_pp[:], s_pp[:])
            nc.sync.dma_start(o_t[:], s_t[:])
            nc.sync.dma_start(o_t2[:], s_t2[:])
    nc.finalize()
    return nc


def _get_nc():
    global _nc_cache
    if _nc_cache is None:
        _nc_cache = _build_bass()
    return _nc_cache


def kernel(input, target, _trace=False):
    x = np.ascontiguousarray(np.asarray(input, dtype=np.float32)).reshape(
        N_CORES, P, FREE
    )
    t = np.ascontiguousarray(np.asarray(target, dtype=np.int32)).reshape(
        N_CORES, P, FREE
    )
    in_maps = [{"input": x[i], "target": t[i]} for i in range(N_CORES)]

    nc = _get_nc()
    res = run_bass_kernel_spmd(
        nc, in_maps, core_ids=list(range(N_CORES)), trace=_trace
    )
    kernel.last_results = res

    s_pt = s_ppt = s_pp = s_t = 0.0
    for r in res.results:
        s_pt += float(r["o_pt"].astype(np.float64).sum())
        s_ppt += float(r["o_ppt"].astype(np.float64).sum())
        s_pp += float(r["o_pp"].astype(np.float64).sum())
        s_t += float(r["o_t"].astype(np.float64).sum())
        s_t += float(r["o_t2"].astype(np.int64).sum())

    c1 = float(s_t)
    c0 = float(TOTAL - s_t)
    w0 = 1.0 / (c0 + SMOOTH) ** 2
    w1 = 1.0 / (c1 + SMOOTH) ** 2
    intersection = w1 * s_pt
    denominator = w0 * (s_pp - s_ppt) + w1 * (s_ppt + c1)
    dice = 1.0 - (2.0 * intersection + SMOOTH) / (denominator + SMOOTH)
    return np.asarray(dice, dtype=np.float32)


# revision 21
# speedup vs baseline: 1.0764x; 1.0585x over previous
"""Balanced dice loss (histogram binning) on 8 Trainium2 NeuronCores.

Math: with t ∈ {0,1} and p = sigmoid(x), the reference loss only needs
four global sums:
    S_t   = Σ t            (count of ones — the bincount)
    S_pt  = Σ p·t
    S_pp  = Σ p²
    S_ppt = Σ p²·t
Then with c1 = S_t, c0 = N − c1, w0 = 1/(c0+s)², w1 = 1/(c1+s)²:
    intersection = w1·S_pt
    denominator  = w0·(S_pp − S_ppt) + w1·(S_ppt + c1)
    dice = 1 − (2·I + s)/(D + s)

Device kernel (data-parallel over 8 cores, batch-sharded), per [128,F] tile:
    ACT : p = sigmoid(x) (bf16); float(t) copy with row-accum → S_t
    DVE : u = p·t, w = u·p (= p²t), sq = p·p   (bf16 products)
    PE  : ones[128,128] @ chunks of u/w/sq → PSUM column-sum accumulation
          (full-array weights so the matmul streams at ~1 col/cycle)
Per-partition/per-tile partials are DMA'd out; host reduces in float64.
"""

import numpy as np

import concourse.bacc as bacc
import concourse.mybir as mybir
from concourse.bass_utils import run_bass_kernel_spmd
from concourse.tile import TileContext

N_CORES = 8
P = 128
TOTAL = 32 * 1024 * 1024  # elements in the full problem
PER_CORE = TOTAL // N_CORES  # 4,194,304
FREE = PER_CORE // P  # 32,768 f32 per partition
F = 4096  # tile free-dim
NT = FREE // F  # tiles per core
MMN = 512  # matmul moving free-dim (one PSUM bank)
FA = 2048  # S_t split: first FA columns summed on ACT, rest on DVE
NCH = F // MMN  # matmul chunks per tile
SMOOTH = 1e-05

_nc_cache = None


def _build_bass():
    nc = bacc.Bacc(None, target_bir_lowering=False)
    x = nc.dram_tensor("input", [P, FREE], mybir.dt.float32, kind="ExternalInput")
    t = nc.dram_tensor("target", [P, FREE], mybir.dt.int32, kind="ExternalInput")
    o_pt = nc.dram_tensor("o_pt", [1, 2 * MMN], mybir.dt.float32, kind="ExternalOutput")
    o_ppt = nc.dram_tensor("o_ppt", [1, 2 * MMN], mybir.dt.float32, kind="ExternalOutput")
    o_pp = nc.dram_tensor("o_pp", [P, NT], mybir.dt.float32, kind="ExternalOutput")
    o_t = nc.dram_tensor("o_t", [P, NT], mybir.dt.float32, kind="ExternalOutput")
    o_t2 = nc.dram_tensor("o_t2", [P, NT], mybir.dt.int32, kind="ExternalOutput")

    with TileContext(nc) as tc:
        with (
            tc.tile_pool(name="work", bufs=2) as pool,
            tc.tile_pool(name="stats", bufs=1) as spool,
            tc.tile_pool(name="ps", bufs=1, space="PSUM") as psum,
        ):
            s_t = spool.tile([P, NT], mybir.dt.float32)
            s_t2 = spool.tile([P, NT], mybir.dt.int32)
            s_pp = spool.tile([P, NT], mybir.dt.float32)
            junk = spool.tile([P, F], mybir.dt.float32, tag="junk")
            ones = spool.tile([P, P], mybir.dt.bfloat16, tag="ones")
            ps_pt_a = psum.tile([P, MMN], mybir.dt.float32, tag="ps_pt_a")
            ps_pt_b = psum.tile([P, MMN], mybir.dt.float32, tag="ps_pt_b")
            ps_ppt_a = psum.tile([P, MMN], mybir.dt.float32, tag="ps_ppt_a")
            ps_ppt_b = psum.tile([P, MMN], mybir.dt.float32, tag="ps_ppt_b")
            nc.any.memset(ones, 1.0)

            for i in range(NT):
                xt = pool.tile([P, F], mybir.dt.float32, tag="xt", bufs=3)
                tt = pool.tile([P, F], mybir.dt.int32, tag="tt", bufs=3)
                pt_ = pool.tile([P, F], mybir.dt.bfloat16, tag="p")
                u = pool.tile([P, F], mybir.dt.bfloat16, tag="u")
                w = pool.tile([P, F], mybir.dt.bfloat16, tag="w")

                nc.sync.dma_start(xt[:], x[:, i * F : (i + 1) * F])
                nc.sync.dma_start(tt[:], t[:, i * F : (i + 1) * F])

                # p = sigmoid(x)                                   [ACT]
                nc.scalar.activation(
                    pt_[:], xt[:], mybir.ActivationFunctionType.Sigmoid
                )
                # u = p·t (bf16·int32), w = u·p                    [DVE]
                nc.vector.tensor_tensor(
                    out=u[:], in0=pt_[:], in1=tt[:], op=mybir.AluOpType.mult
                )
                nc.vector.tensor_tensor(
                    out=w[:], in0=u[:], in1=pt_[:], op=mybir.AluOpType.mult
                )
                # S_pp row-accum via p² (junk sink)                [ACT]
                nc.scalar.activation(
                    junk[:],
                    pt_[:],
                    mybir.ActivationFunctionType.Square,
                    accum_out=s_pp[:, i : i + 1],
                )
                # S_t split: float(t) copy+accum on [:FA]          [ACT]
                nc.scalar.activation(
                    junk[:, :FA],
                    tt[:, :FA],
                    mybir.ActivationFunctionType.Copy,
                    accum_out=s_t[:, i : i + 1],
                )
                # ... and int32 row-reduce on [FA:] (exact)         [DVE]
                with nc.allow_low_precision("int32 add is exact"):
                    nc.vector.tensor_reduce(
                        s_t2[:, i : i + 1],
                        tt[:, FA:],
                        axis=mybir.AxisListType.X,
                        op=mybir.AluOpType.add,
                    )
                # column-sum accumulation of u and w; each chain
                # alternates two PSUM banks to pipeline the RMW    [PE]
                for s_, banks in ((u, (ps_pt_a, ps_pt_b)), (w, (ps_ppt_a, ps_ppt_b))):
                    for j in range(NCH):
                        nc.tensor.matmul(
                            banks[j % 2][:],
                            ones[:],
                            s_[:, j * MMN : (j + 1) * MMN],
                            start=(i == 0 and j < 2),
                            stop=(i == NT - 1 and j >= NCH - 2),
                        )

            fin = spool.tile([1, 4 * MMN], mybir.dt.float32, tag="fin")
            nc.vector.tensor_copy(fin[:, 0:MMN], ps_pt_a[0:1, :])
            nc.vector.tensor_copy(fin[:, MMN : 2 * MMN], ps_pt_b[0:1, :])
            nc.vector.tensor_copy(fin[:, 2 * MMN : 3 * MMN], ps_ppt_a[0:1, :])
            nc.vector.tensor_copy(fin[:, 3 * MMN :], ps_ppt_b[0:1, :])
            nc.sync.dma_start(o_pt[:], fin[:, 0 : 2 * MMN])
            nc.sync.dma_start(o_ppt[:], fin[:, 2 * MMN :])
            nc.sync.dma_start(o_pp[:], s_pp[:])
            nc.sync.dma_start(o_t[:], s_t[:])
            nc.sync.dma_start(o_t2[:], s_t2[:])
    nc.finalize()
    return nc


def _get_nc():
    global _nc_cache
    if _nc_cache is None:
        _nc_cache = _build_bass()
    return _nc_cache


def kernel(input, target, _trace=False):
    x = np.ascontiguousarray(np.asarray(input, dtype=np.float32)).reshape(
        N_CORES, P, FREE
    )
    t = np.ascontiguousarray(np.asarray(target, dtype=np.int32)).reshape(
        N_CORES, P, FREE
    )
    in_maps = [{"input": x[i], "target": t[i]} for i in range(N_CORES)]

    nc = _get_nc()
    res = run_bass_kernel_spmd(
        nc, in_maps, core_ids=list(range(N_CORES)), trace=_trace
    )
    kernel.last_results = res

    s_pt = s_ppt = s_pp = s_t = 0.0
    for r in res.results:
        s_pt += float(r["o_pt"].astype(np.float64).sum())
        s_ppt += float(r["o_ppt"].astype(np.float64).sum())
        s_pp += float(r["o_pp"].astype(np.float64).sum())
        s_t += float(r["o_t"].astype(np.float64).sum())
        s_t += float(r["o_t2"].astype(np.int64).sum())

    c1 = float(s_t)
    c0 = float(TOTAL - s_t)
    w0 = 1.0 / (c0 + SMOOTH) ** 2
    w1 = 1.0 / (c1 + SMOOTH) ** 2
    intersection = w1 * s_pt
    denominator = w0 * (s_pp - s_ppt) + w1 * (s_ppt + c1)
    dice = 1.0 - (2.0 * intersection + SMOOTH) / (denominator + SMOOTH)
    return np.asarray(dice, dtype=np.float32)


# revision 23
# speedup vs baseline: 1.1550x; 1.0730x over previous
"""Balanced dice loss (histogram binning) on 8 Trainium2 NeuronCores.

Math: with t ∈ {0,1} and p = sigmoid(x), the reference loss only needs
four global sums:
    S_t   = Σ t            (count of ones — the bincount)
    S_pt  = Σ p·t
    S_pp  = Σ p²
    S_ppt = Σ p²·t
Then with c1 = S_t, c0 = N − c1, w0 = 1/(c0+s)², w1 = 1/(c1+s)²:
    intersection = w1·S_pt
    denominator  = w0·(S_pp − S_ppt) + w1·(S_ppt + c1)
    dice = 1 − (2·I + s)/(D + s)

Device kernel (data-parallel over 8 cores, batch-sharded), per [128,F] tile:
    ACT : p = sigmoid(x) (bf16 out); p² with row-accum → S_pp;
          float(t) copy on the first FA columns with row-accum → S_t part 1
    DVE : u = p·t (int32 converts in-pipe), w = u·p (= p²·t, bf16 2x mode);
          int32 row-reduce of t on the remaining columns → S_t part 2
    PE  : ones[128,128] @ chunks of u and w → PSUM column-sum accumulation
          (each chain alternates two PSUM banks; S_pt, S_ppt)
The work is split so ACT/DVE/PE all sit just under the ~94 µs DMA
roofline (32 MB/core at ~358 GB/s per-core HBM bandwidth).
Per-partition/per-tile partials are DMA'd out; host reduces in float64.
"""

import numpy as np

import concourse.bacc as bacc
import concourse.mybir as mybir
from concourse.bass_utils import run_bass_kernel_spmd
from concourse.tile import TileContext

N_CORES = 8
P = 128
TOTAL = 32 * 1024 * 1024  # elements in the full problem
PER_CORE = TOTAL // N_CORES  # 4,194,304
FREE = PER_CORE // P  # 32,768 f32 per partition
F = 4096  # tile free-dim
NT = FREE // F  # tiles per core
MMN = 512  # matmul moving free-dim (one PSUM bank)
FA = 2048  # S_t split: first FA columns summed on ACT, rest on DVE
NCH = F // MMN  # matmul chunks per tile
SMOOTH = 1e-05

_nc_cache = None


def _build_bass():
    nc = bacc.Bacc(None, target_bir_lowering=False)
    x = nc.dram_tensor("input", [P, FREE], mybir.dt.float32, kind="ExternalInput")
    t = nc.dram_tensor("target", [P, FREE], mybir.dt.int32, kind="ExternalInput")
    o_pt = nc.dram_tensor("o_pt", [1, 2 * MMN], mybir.dt.float32, kind="ExternalOutput")
    o_ppt = nc.dram_tensor("o_ppt", [1, 2 * MMN], mybir.dt.float32, kind="ExternalOutput")
    o_pp = nc.dram_tensor("o_pp", [P, NT], mybir.dt.float32, kind="ExternalOutput")
    o_t = nc.dram_tensor("o_t", [P, NT], mybir.dt.float32, kind="ExternalOutput")
    o_t2 = nc.dram_tensor("o_t2", [P, NT], mybir.dt.int32, kind="ExternalOutput")

    with TileContext(nc) as tc:
        with (
            tc.tile_pool(name="work", bufs=2) as pool,
            tc.tile_pool(name="stats", bufs=1) as spool,
            tc.tile_pool(name="ps", bufs=1, space="PSUM") as psum,
        ):
            s_t = spool.tile([P, NT], mybir.dt.float32)
            s_t2 = spool.tile([P, NT], mybir.dt.int32)
            s_pp = spool.tile([P, NT], mybir.dt.float32)
            junk = spool.tile([P, F], mybir.dt.float32, tag="junk")
            ones = spool.tile([P, P], mybir.dt.bfloat16, tag="ones")
            ps_pt_a = psum.tile([P, MMN], mybir.dt.float32, tag="ps_pt_a")
            ps_pt_b = psum.tile([P, MMN], mybir.dt.float32, tag="ps_pt_b")
            ps_ppt_a = psum.tile([P, MMN], mybir.dt.float32, tag="ps_ppt_a")
            ps_ppt_b = psum.tile([P, MMN], mybir.dt.float32, tag="ps_ppt_b")
            nc.any.memset(ones, 1.0)

            for i in range(NT):
                xt = pool.tile([P, F], mybir.dt.float32, tag="xt", bufs=3)
                tt = pool.tile([P, F], mybir.dt.int32, tag="tt", bufs=3)
                pt_ = pool.tile([P, F], mybir.dt.bfloat16, tag="p")
                u = pool.tile([P, F], mybir.dt.bfloat16, tag="u")
                w = pool.tile([P, F], mybir.dt.bfloat16, tag="w")

                nc.sync.dma_start(xt[:], x[:, i * F : (i + 1) * F])
                nc.sync.dma_start(tt[:], t[:, i * F : (i + 1) * F])

                # int32 row-reduce of t on [FA:] (exact): S_t pt 2 [DVE]
                with nc.allow_low_precision("int32 add is exact"):
                    nc.vector.tensor_reduce(
                        s_t2[:, i : i + 1],
                        tt[:, FA:],
                        axis=mybir.AxisListType.X,
                        op=mybir.AluOpType.add,
                    )

                # p = sigmoid(x)                                   [ACT]
                nc.scalar.activation(
                    pt_[:], xt[:], mybir.ActivationFunctionType.Sigmoid
                )
                # u = p·t (bf16·int32), w = u·p                    [DVE]
                nc.vector.tensor_tensor(
                    out=u[:], in0=pt_[:], in1=tt[:], op=mybir.AluOpType.mult
                )
                nc.vector.tensor_tensor(
                    out=w[:], in0=u[:], in1=pt_[:], op=mybir.AluOpType.mult
                )
                # S_pp row-accum via p² (junk sink)                [ACT]
                nc.scalar.activation(
                    junk[:],
                    pt_[:],
                    mybir.ActivationFunctionType.Square,
                    accum_out=s_pp[:, i : i + 1],
                )
                # S_t split: float(t) copy+accum on [:FA]          [ACT]
                nc.scalar.activation(
                    junk[:, :FA],
                    tt[:, :FA],
                    mybir.ActivationFunctionType.Copy,
                    accum_out=s_t[:, i : i + 1],
                )
                # column-sum accumulation of u and w; each chain
                # alternates two PSUM banks to pipeline the RMW    [PE]
                for s_, banks in ((u, (ps_pt_a, ps_pt_b)), (w, (ps_ppt_a, ps_ppt_b))):
                    for j in range(NCH):
                        nc.tensor.matmul(
                            banks[j % 2][:],
                            ones[:],
                            s_[:, j * MMN : (j + 1) * MMN],
                            start=(i == 0 and j < 2),
                            stop=(i == NT - 1 and j >= NCH - 2),
                        )

            fin = spool.tile([1, 4 * MMN], mybir.dt.float32, tag="fin")
            nc.vector.tensor_copy(fin[:, 0:MMN], ps_pt_a[0:1, :])
            nc.vector.tensor_copy(fin[:, MMN : 2 * MMN], ps_pt_b[0:1, :])
            nc.vector.tensor_copy(fin[:, 2 * MMN : 3 * MMN], ps_ppt_a[0:1, :])
            nc.vector.tensor_copy(fin[:, 3 * MMN :], ps_ppt_b[0:1, :])
            nc.sync.dma_start(o_pt[:], fin[:, 0 : 2 * MMN])
            nc.sync.dma_start(o_ppt[:], fin[:, 2 * MMN :])
            nc.sync.dma_start(o_pp[:], s_pp[:])
            nc.sync.dma_start(o_t[:], s_t[:])
            nc.sync.dma_start(o_t2[:], s_t2[:])
    nc.finalize()
    return nc


def _get_nc():
    global _nc_cache
    if _nc_cache is None:
        _nc_cache = _build_bass()
    return _nc_cache


def kernel(input, target, _trace=False):
    x = np.ascontiguousarray(np.asarray(input, dtype=np.float32)).reshape(
        N_CORES, P, FREE
    )
    t = np.ascontiguousarray(np.asarray(target, dtype=np.int32)).reshape(
        N_CORES, P, FREE
    )
    in_maps = [{"input": x[i], "target": t[i]} for i in range(N_CORES)]

    nc = _get_nc()
    res = run_bass_kernel_spmd(
        nc, in_maps, core_ids=list(range(N_CORES)), trace=_trace
    )
    kernel.last_results = res

    s_pt = s_ppt = s_pp = s_t = 0.0
    for r in res.results:
        s_pt += float(r["o_pt"].astype(np.float64).sum())
        s_ppt += float(r["o_ppt"].astype(np.float64).sum())
        s_pp += float(r["o_pp"].astype(np.float64).sum())
        s_t += float(r["o_t"].astype(np.float64).sum())
        s_t += float(r["o_t2"].astype(np.int64).sum())

    c1 = float(s_t)
    c0 = float(TOTAL - s_t)
    w0 = 1.0 / (c0 + SMOOTH) ** 2
    w1 = 1.0 / (c1 + SMOOTH) ** 2
    intersection = w1 * s_pt
    denominator = w0 * (s_pp - s_ppt) + w1 * (s_ppt + c1)
    dice = 1.0 - (2.0 * intersection + SMOOTH) / (denominator + SMOOTH)
    return np.asarray(dice, dtype=np.float32)
